# revision 8
# baseline (speedup 1.0000x reference)
"""Distributed Trainium2 kernel for relative-position causal attention.

N=M=2048, B=1, D=1024, H=16, DQK=DV=64, OFFSET=0.
2 heads per core on 8 NeuronCores. Per core:
  - projections: Q/K/PErev in transposed layout [dqk2, n], V natural [m, dv]
    with an appended ones column (so the ctx matmul also yields the softmax
    denominator, already transposed)
  - per 128-row block: S = Q K^T + skew(Q PErev^T); the rel-shift gather is an
    SBUF->SBUF DMA whose source row stride is (width-1), i.e. a diagonal read
  - causal mask add, exp without max subtraction (logits are bounded), P bf16
  - PE-transpose of P tiles, ctx^T accumulated in PSUM
  - AllGather of ctx^T (f32), then each core computes a 256-row slice of
    out = ctx @ to_out^T; host concatenates slices.
"""

import sys

sys.path.insert(0, "/opt/trn_rl_repo")

import numpy as np
import ml_dtypes

from concourse import bass, bacc, tile, mybir
from concourse.ap import AP
from concourse.bass_utils import run_bass_kernel_spmd

N, M, D, H, DQK, DV = 2048, 2048, 1024, 16, 64, 64
RP = 2048
NCORES = 8
NB = N // 128
KT = D // 128
F2 = RP + 128
VW = DV + 1
SLICE = N // NCORES

BF = mybir.dt.bfloat16
F32 = mybir.dt.float32
RG = [list(range(NCORES))]

_cache = {}


def _build():
    nc = bacc.Bacc("TRN2", target_bir_lowering=False, debug=False,
                   num_devices=NCORES)

    xqT = nc.dram_tensor("xqT", [D, N], BF, kind="ExternalInput")
    xkvT = nc.dram_tensor("xkvT", [D, M], BF, kind="ExternalInput")
    scT = nc.dram_tensor("scT", [D, RP], BF, kind="ExternalInput")
    wqT = nc.dram_tensor("wqT", [D, 128], BF, kind="ExternalInput")
    wkT = nc.dram_tensor("wkT", [D, 128], BF, kind="ExternalInput")
    wvT = nc.dram_tensor("wvT", [D, 128], BF, kind="ExternalInput")
    fpeT = nc.dram_tensor("fpeT", [D, 128], BF, kind="ExternalInput")
    woT = nc.dram_tensor("woT", [H * DV, D], BF, kind="ExternalInput")
    maskc = nc.dram_tensor("maskc", [128, 128], F32, kind="ExternalInput")
    identc = nc.dram_tensor("identc", [128, 128], BF, kind="ExternalInput")
    out_ext = nc.dram_tensor("out", [SLICE, D], F32, kind="ExternalOutput")

    cc_in = nc.dram_tensor("cc_in", [128 * NCORES, SLICE], F32)
    cc_out = nc.dram_tensor("cc_out", [128 * NCORES, SLICE], F32)

    with tile.TileContext(nc) as tc:
        with (
            tc.tile_pool(name="const", bufs=1) as cpool,
            tc.tile_pool(name="proj", bufs=1) as proj,
            tc.tile_pool(name="xload", bufs=2) as xload,
            tc.tile_pool(name="work", bufs=2) as work,
            tc.tile_pool(name="small", bufs=4) as small,
        ):
            # ---- constants
            wq_sb = cpool.tile([128, KT, 128], BF, tag="wq")
            wk_sb = cpool.tile([128, KT, 128], BF, tag="wk")
            wv_sb = cpool.tile([128, KT, 128], BF, tag="wv")
            fpe_sb = cpool.tile([128, KT, 128], BF, tag="fpe")
            for dst, src in ((wq_sb, wqT), (wk_sb, wkT), (wv_sb, wvT),
                             (fpe_sb, fpeT)):
                nc.sync.dma_start(
                    dst[:], src.ap().rearrange("(k p) c -> p k c", p=128))
            wo_sb = cpool.tile([128, KT, D], BF, tag="wo")
            nc.sync.dma_start(
                wo_sb[:], woT.ap().rearrange("(k p) c -> p k c", p=128))
            mask_sb = cpool.tile([128, 128], F32, tag="mask")
            nc.sync.dma_start(mask_sb[:], maskc[:])
            ident = cpool.tile([128, 128], BF, tag="ident")
            nc.sync.dma_start(ident[:], identc[:])

            # ---- persistent activations
            q2T = proj.tile([128, N], BF, tag="q2T")
            k2T = proj.tile([128, M], BF, tag="k2T")
            pe2T = proj.tile([128, RP], BF, tag="pe2T")
            v2 = proj.tile([128, NB, 128], BF, tag="v2")
            ctxh = [proj.tile([64, N], F32, tag=f"ctxh{h}", name=f"ctxh{h}")
                    for h in range(2)]

            # ---- projections (scoped pools; freed before attention)
            with (
                tc.tile_pool(name="xkvp", bufs=1) as xkvp,
                tc.tile_pool(name="xstream", bufs=3) as xstream,
                tc.tile_pool(name="psP", bufs=1, space="PSUM") as psP,
            ):
                xkv_t = []
                for k in range(KT):
                    t = xkvp.tile([128, M], BF, tag=f"xkv{k}")
                    nc.sync.dma_start(t[:], xkvT[k * 128:(k + 1) * 128, :])
                    xkv_t.append(t)

                # K from resident xkv tiles (psum chunk accumulation)
                for ch in range(M // 512):
                    ps = psP.tile([128, 512], F32, tag=f"hacc{ch}",
                                  name=f"kacc{ch}")
                    for k in range(KT):
                        nc.tensor.matmul(ps[:], wk_sb[:, k, :],
                                         xkv_t[k][:, ch * 512:(ch + 1) * 512],
                                         start=(k == 0), stop=(k == KT - 1))
                    nc.scalar.activation(k2T[:, ch * 512:(ch + 1) * 512],
                                         ps[:],
                                         mybir.ActivationFunctionType.Copy)
                # V natural layout
                for mt in range(NB):
                    ps = psP.tile([128, 128], F32, tag="projv", bufs=2,
                                  name=f"vacc{mt}")
                    for k in range(KT):
                        nc.tensor.matmul(
                            ps[:], xkv_t[k][:, mt * 128:(mt + 1) * 128],
                            wv_sb[:, k, :],
                            start=(k == 0), stop=(k == KT - 1))
                    nc.scalar.activation(v2[:, mt, :], ps[:],
                                         mybir.ActivationFunctionType.Copy)
                # Q and PErev: stream x tiles, accumulate in 4 held banks
                for which, xdram, wtile, dest in (
                    ("q", xqT, wq_sb, q2T), ("pe", scT, fpe_sb, pe2T),
                ):
                    accs = [psP.tile([128, 512], F32, tag=f"hacc{ch}",
                                     name=f"{which}acc{ch}")
                            for ch in range(4)]
                    for k in range(KT):
                        t = xstream.tile([128, N], BF, tag="xs",
                                         name=f"{which}x{k}")
                        nc.sync.dma_start(t[:], xdram[k * 128:(k + 1) * 128, :])
                        for ch in range(4):
                            nc.tensor.matmul(
                                accs[ch][:], wtile[:, k, :],
                                t[:, ch * 512:(ch + 1) * 512],
                                start=(k == 0), stop=(k == KT - 1))
                    for ch in range(4):
                        nc.scalar.activation(
                            dest[:, ch * 512:(ch + 1) * 512], accs[ch][:],
                            mybir.ActivationFunctionType.Copy)

            # ---- attention
            with (
                tc.tile_pool(name="psS", bufs=2, space="PSUM") as psS,
                tc.tile_pool(name="psR", bufs=2, space="PSUM") as psR,
                tc.tile_pool(name="psT", bufs=2, space="PSUM") as psT,
                tc.tile_pool(name="psX", bufs=2, space="PSUM") as psX,
            ):
                for hl in range(2):
                    hb = hl * 64
                    for nb in range(NB):
                        n0 = nb * 128
                        span = n0 + 128
                        nch = (span + 511) // 512
                        c_lo = (RP - 1 - n0 - 127) // 512
                        plr = work.tile([128, F2], F32, tag="plr")
                        nc.vector.memset(plr[:, RP:F2], 0.0)
                        for ch in range(c_lo, RP // 512):
                            ps = psR.tile([128, 512], F32, tag="plrev")
                            nc.tensor.matmul(
                                ps[:],
                                q2T[hb:hb + 64, n0:n0 + 128],
                                pe2T[hb:hb + 64, ch * 512:(ch + 1) * 512],
                                start=True, stop=True)
                            nc.scalar.activation(
                                plr[:, ch * 512:(ch + 1) * 512], ps[:],
                                mybir.ActivationFunctionType.Copy)
                        pos = work.tile([128, span], F32, tag="pos")
                        src = AP(plr[:].tensor, plr[:].offset + (RP - 1 - n0),
                                 [[F2 - 1, 128], [1, span]])
                        nc.sync.dma_start(pos[:], src)

                        sS = work.tile([128, span], F32, tag="sS")
                        for ch in range(nch):
                            cw = min(512, span - ch * 512)
                            ps = psS.tile([128, 512], F32, tag="cont")
                            nc.tensor.matmul(
                                ps[:, :cw],
                                q2T[hb:hb + 64, n0:n0 + 128],
                                k2T[hb:hb + 64, ch * 512:ch * 512 + cw],
                                start=True, stop=True)
                            nc.vector.tensor_tensor(
                                sS[:, ch * 512:ch * 512 + cw], ps[:, :cw],
                                pos[:, ch * 512:ch * 512 + cw],
                                mybir.AluOpType.add)
                        nc.vector.tensor_tensor(
                            sS[:, n0:n0 + 128], sS[:, n0:n0 + 128],
                            mask_sb[:], mybir.AluOpType.add)
                        pP = work.tile([128, span], BF, tag="pP")
                        lrow = small.tile([128, 1], F32, tag="lrow")
                        nc.scalar.activation(pP[:], sS[:],
                                             mybir.ActivationFunctionType.Exp,
                                             accum_out=lrow[:])
                        linv = small.tile([128, 1], F32, tag="linv")
                        nc.vector.reciprocal(linv[:], lrow[:])
                        diagt = small.tile([128, 128], BF, tag="diagt")
                        nc.vector.tensor_scalar_mul(diagt[:], ident[:],
                                                    linv[:])
                        ctxp = psX.tile([64, 128], F32, tag="ctx")
                        for mt in range(nb + 1):
                            pt_ps = psT.tile([128, 128], F32, tag="ptT")
                            nc.tensor.matmul(
                                pt_ps[:], pP[:, mt * 128:(mt + 1) * 128],
                                diagt[:], start=True, stop=True)
                            pt_sb = small.tile([128, 128], BF, tag="ptsb")
                            nc.any.tensor_copy(pt_sb[:], pt_ps[:])
                            nc.tensor.matmul(
                                ctxp[:], v2[:, mt, hl * 64:hl * 64 + 64],
                                pt_sb[:],
                                start=(mt == 0), stop=(mt == nb))
                        nc.vector.tensor_copy(ctxh[hl][:, n0:n0 + 128],
                                              ctxp[:])

            # ---- collective: AllToAll. cc_in row-block j (128 rows) holds
            # ctx2T[:, 256j:256j+256]; rank p receives [1024ch, 256n(slice p)]
            for hl in range(2):
                dst = AP(cc_in, hl * 64 * SLICE,
                         [[SLICE, 64], [128 * SLICE, NCORES], [1, SLICE]])
                src_ap = ctxh[hl][:]
                src_ap = AP(src_ap.tensor, src_ap.offset,
                            [[N, 64], [SLICE, NCORES], [1, SLICE]])
                nc.sync.dma_start(dst, src_ap)
            nc.gpsimd.collective_compute(
                "AllToAll",
                mybir.AluOpType.bypass,
                ins=[cc_in[:]],
                outs=[cc_out[:]],
                replica_groups=RG,
            )

            # ---- out projection on the received slice
            stages = [small.tile([128, SLICE], F32, tag=f"ccst{k % 2}",
                                 name=f"ccst{k}") for k in range(KT)]
            for k in range(KT):
                nc.sync.dma_start(stages[k][:],
                                  cc_out[k * 128:(k + 1) * 128, :])
            ctxall = proj.tile([128, KT, SLICE], BF, tag="ctxall")
            for k in range(KT):
                nc.vector.tensor_copy(ctxall[:, k, :], stages[k][:])
            with tc.tile_pool(name="psO", bufs=2, space="PSUM") as psO:
                for nt in range(SLICE // 128):
                    for dc in range(D // 512):
                        ps = psO.tile([128, 512], F32, tag="out")
                        for k in range(KT):
                            nc.tensor.matmul(
                                ps[:],
                                ctxall[:, k, nt * 128:(nt + 1) * 128],
                                wo_sb[:, k, dc * 512:(dc + 1) * 512],
                                start=(k == 0), stop=(k == KT - 1))
                        ostage = small.tile([128, 512], F32, tag="ostage")
                        nc.scalar.activation(
                            ostage[:], ps[:],
                            mybir.ActivationFunctionType.Copy)
                        nc.sync.dma_start(
                            out_ext[nt * 128:(nt + 1) * 128,
                                    dc * 512:(dc + 1) * 512], ostage[:])

    nc.compile()
    return nc


def _host_prep(inputs):
    bf16 = ml_dtypes.bfloat16
    x_q = np.asarray(inputs["x_q"])[:, 0, :]
    x_kv = np.asarray(inputs["x_kv"])[:, 0, :]
    to_q = np.asarray(inputs["to_q"])
    to_k = np.asarray(inputs["to_k"])
    to_v = np.asarray(inputs["to_v"])
    to_out = np.asarray(inputs["to_out"])
    fpe = np.asarray(inputs["for_pos_enc"])

    xqT = np.ascontiguousarray(x_q.T).astype(bf16)
    xkvT = np.ascontiguousarray(x_kv.T).astype(bf16)

    r = np.arange(0, RP, dtype=np.float32)
    inv_freq = 1.0 / (10000.0 ** (np.arange(0.0, D, 2.0, np.float32) / D))
    ph = r[:, None] * inv_freq[None, :]
    sincos = np.concatenate([np.sin(ph), np.cos(ph)], axis=-1)
    scT = np.ascontiguousarray(sincos[::-1].T).astype(bf16)

    mask = np.triu(np.full((128, 128), -1e30, np.float32), 1)
    identity = np.eye(128, dtype=bf16)
    woT = np.ascontiguousarray(
        to_out.transpose(0, 2, 1).reshape(D, H * DV).T).astype(bf16)

    in_maps = []
    for c in range(NCORES):
        hs = [2 * c, 2 * c + 1]
        in_maps.append({
            "xqT": xqT, "xkvT": xkvT, "scT": scT,
            "wqT": np.ascontiguousarray(
                np.concatenate([to_q[:, h, :].T for h in hs], 1)).astype(bf16),
            "wkT": np.ascontiguousarray(
                np.concatenate([to_k[:, h, :].T for h in hs], 1)).astype(bf16),
            "wvT": np.ascontiguousarray(
                np.concatenate([to_v[:, h, :].T for h in hs], 1)).astype(bf16),
            "fpeT": np.ascontiguousarray(
                np.concatenate([fpe[:, h, :].T for h in hs], 1)).astype(bf16),
            "woT": woT, "maskc": mask, "identc": identity,
        })
    return in_maps


def kernel(**inputs):
    if "nc" not in _cache:
        _cache["nc"] = _build()
    nc = _cache["nc"]
    in_maps = _host_prep(inputs)
    res = run_bass_kernel_spmd(nc, in_maps, list(range(NCORES)))
    out = np.concatenate([res.results[c]["out"] for c in range(NCORES)], 0)
    return out.reshape(N, 1, D).astype(np.float32)


if __name__ == "__main__":
    import pickle
    with open("/tmp/inputs.pkl", "rb") as f:
        inputs = pickle.load(f)
    out = kernel(**inputs)
    exp = np.load("/tmp/expected.npy")
    err = np.linalg.norm(out - exp) / np.linalg.norm(exp)
    print("Relative error:", err)
